# revision 1
# baseline (speedup 1.0000x reference)
"""Izhikevich 2-layer SNN kernel for 8 Trainium2 NeuronCores.

Reference computation (per timestep t of 100):
    cur1 = x_t @ W1.T + b1                 # [B, 100]
    spk1, v1, u1 = izh(cur1, v1, u1)
    cur2 = spk1 @ W2.T + b2                # [B, 10]
    spk2, v2, u2 = izh(cur2, v2, u2)
    record spk2, v2
Output: (spk2_rec, mem2_rec), each [100, B, 10].

Sharding: pure data parallel over batch (2048 -> 8 x 256), weights replicated.

Device layout: feature-on-partition.  Host pre-transposes x to
[T/TB, 112, TB*7*256] per core so each DMA is a fully-contiguous slab and the
784-dim contraction maps to 7 chained matmuls of K=112 on the PE
(out[100,256] += W1T_k.T @ x_k).  States v,u live as [feat, batch] tiles.

Izhikevich algebra used on device (exactly equivalent in exact arithmetic):
    v_new = v + 0.04 v^2 + 5v + 140 - u + I = 0.04(v+75)^2 - 85 + (I_mm + beta - u)
          = Square(0.2 v + 15.0) + (I_mm - ubar),   ubar := u + 85 - beta
    ubar' = (1-a) ubar + (a*b) v + a*(85 - beta)
    spk   = v_new >= thr;  v = spk ? c : v_new;  ubar += d*spk
where beta is the layer bias (b1/b2) folded into the shifted state ubar.
"""

import os
from contextlib import ExitStack

import numpy as np

import concourse.bass as bass
import concourse.bacc as bacc
import concourse.mybir as mybir
import concourse.tile as tile
from concourse.bass_utils import run_bass_kernel_spmd

# Izhikevich RS config + threshold (matches reference.py)
A_, B_, C_, D_ = 0.02, 0.2, -65.0, 8.0
THR = 0.03

T, F, H, O = 100, 784, 100, 10
P, KC = 112, 7  # F == KC * P
NCORES = 8
BATCH = 2048
BC = BATCH // NCORES  # 256 batch per core

TB = 2     # timesteps per x DMA (1.6 MB per dma_start)
FLUSH = 25  # timesteps staged in SBUF between output DMAs

LAST_RUN = None  # BassKernelResults of the most recent kernel() call


def build_program(nc, ctx, tc, Bc=BC, T_=T, TB_=TB, FLUSH_=FLUSH):
    f32 = mybir.dt.float32
    f32r = mybir.dt.float32r
    AL = mybir.AluOpType
    AF = mybir.ActivationFunctionType

    xT = nc.dram_tensor("xT", [T_ // TB_, P, TB_ * KC * Bc], f32r, kind="ExternalInput").ap()
    w1 = nc.dram_tensor("w1t", [P, KC * H], f32r, kind="ExternalInput").ap()
    w2 = nc.dram_tensor("w2t", [H, O], f32r, kind="ExternalInput").ap()
    u1i = nc.dram_tensor("u1i", [H, Bc], f32, kind="ExternalInput").ap()
    u2i = nc.dram_tensor("u2i", [O, Bc], f32, kind="ExternalInput").ap()
    g1 = nc.dram_tensor("g1", [H, 1], f32, kind="ExternalInput").ap()
    g2 = nc.dram_tensor("g2", [O, 1], f32, kind="ExternalInput").ap()
    out = nc.dram_tensor("out", [2, O, T_, Bc], f32, kind="ExternalOutput").ap()

    const = ctx.enter_context(tc.tile_pool(name="const", bufs=1))
    state = ctx.enter_context(tc.tile_pool(name="state", bufs=1))
    xpool = ctx.enter_context(tc.tile_pool(name="x", bufs=3))
    s1pool = ctx.enter_context(tc.tile_pool(name="scr1", bufs=2))
    s2pool = ctx.enter_context(tc.tile_pool(name="scr2", bufs=2))
    stpool = ctx.enter_context(tc.tile_pool(name="stage", bufs=2))
    pp1 = ctx.enter_context(tc.tile_pool(name="ps1", bufs=4, space="PSUM"))
    pp2 = ctx.enter_context(tc.tile_pool(name="ps2", bufs=2, space="PSUM"))

    w1sb = const.tile([P, KC * H], f32r)
    nc.sync.dma_start(w1sb[:], w1)
    w2sb = const.tile([H, O], f32r)
    nc.sync.dma_start(w2sb[:], w2)
    g1sb = const.tile([H, 1], f32)
    nc.sync.dma_start(g1sb[:], g1)
    g2sb = const.tile([O, 1], f32)
    nc.sync.dma_start(g2sb[:], g2)
    cc = const.tile([H, Bc], f32)
    nc.vector.memset(cc[:], C_)
    b125 = const.tile([H, 1], f32)
    nc.vector.memset(b125[:], 15.0)

    v1 = state.tile([H, Bc], f32)
    nc.vector.memset(v1[:], -70.0)
    u1 = state.tile([H, Bc], f32)
    nc.sync.dma_start(u1[:], u1i)
    u2 = state.tile([O, Bc], f32)
    nc.sync.dma_start(u2[:], u2i)
    v2prev = state.tile([O, Bc], f32)
    nc.vector.memset(v2prev[:], -70.0)
    v2prev = v2prev[:]

    stage_s = stage_v = None
    for tb in range(T_ // TB_):
        xt = xpool.tile([P, TB_ * KC * Bc], f32r)
        nc.sync.dma_start(xt[:], xT[tb, :, :])
        # layer-1 matmuls for all timesteps of this DMA first (keeps PE fed)
        p1s = []
        for tt in range(TB_):
            p1 = pp1.tile([H, Bc], f32)
            for k in range(KC):
                nc.tensor.matmul(
                    p1[:],
                    w1sb[:, k * H:(k + 1) * H],
                    xt[:, (tt * KC + k) * Bc:(tt * KC + k + 1) * Bc],
                    start=(k == 0),
                    stop=(k == KC - 1),
                )
            p1s.append(p1)
        for tt in range(TB_):
            t = tb * TB_ + tt
            tm = t % FLUSH_
            if tm == 0:
                stage_s = stpool.tile([O, FLUSH_ * Bc], f32, tag="ss")
                stage_v = stpool.tile([O, FLUSH_ * Bc], f32, tag="sv")
            p1 = p1s[tt]
            # ---- izhikevich layer 1 on [100, Bc] ----
            q1 = s1pool.tile([H, Bc], f32, tag="q1")
            z1 = s1pool.tile([H, Bc], f32, tag="z1")
            wv1 = s1pool.tile([H, Bc], f32, tag="wv1")
            sp1 = s1pool.tile([H, Bc], f32r, tag="sp1")
            nc.scalar.activation(q1[:], v1[:], AF.Square, bias=b125[:, 0:1], scale=0.2)
            nc.scalar.activation(z1[:], v1[:], AF.Identity, bias=g1sb[:, 0:1], scale=A_ * B_)
            nc.vector.tensor_tensor(wv1[:], p1[:], u1[:], AL.subtract)
            nc.vector.tensor_tensor(v1[:], q1[:], wv1[:], AL.add)  # v1 <- v_new
            nc.vector.tensor_scalar(sp1[:], v1[:], THR, None, AL.is_ge)
            nc.vector.copy_predicated(v1[:], sp1[:].bitcast(mybir.dt.uint32), cc[:])
            nc.vector.scalar_tensor_tensor(u1[:], u1[:], 1.0 - A_, z1[:], AL.mult, AL.add)
            nc.vector.scalar_tensor_tensor(u1[:], sp1[:], D_, u1[:], AL.mult, AL.add)
            # ---- layer 2 matmul: cur2 = W2T.T @ spk1 ----
            p2 = pp2.tile([O, Bc], f32)
            nc.tensor.matmul(p2[:], w2sb[:, :], sp1[:],
                             start=True, stop=True)
            # ---- izhikevich layer 2 on [10, Bc], v2 state lives in stage_v ----
            svcol = stage_v[:, tm * Bc:(tm + 1) * Bc]
            sscol = stage_s[:, tm * Bc:(tm + 1) * Bc]
            q2 = s2pool.tile([O, Bc], f32, tag="q2")
            z2 = s2pool.tile([O, Bc], f32, tag="z2")
            wv2 = s2pool.tile([O, Bc], f32, tag="wv2")
            nc.scalar.activation(q2[:], v2prev, AF.Square, bias=b125[0:O, 0:1], scale=0.2)
            nc.scalar.activation(z2[:], v2prev, AF.Identity, bias=g2sb[:, 0:1], scale=A_ * B_)
            nc.vector.tensor_tensor(wv2[:], p2[:], u2[:], AL.subtract)
            nc.vector.tensor_tensor(svcol, q2[:], wv2[:], AL.add)
            nc.vector.tensor_scalar(sscol, svcol, THR, None, AL.is_ge)
            nc.vector.copy_predicated(svcol, sscol.bitcast(mybir.dt.uint32), cc[0:O, :])
            nc.vector.scalar_tensor_tensor(u2[:], u2[:], 1.0 - A_, z2[:], AL.mult, AL.add)
            nc.vector.scalar_tensor_tensor(u2[:], sscol, D_, u2[:], AL.mult, AL.add)
            v2prev = svcol
            if tm == FLUSH_ - 1:
                c0 = t - (FLUSH_ - 1)
                nc.sync.dma_start(
                    out[0, :, c0:t + 1, :],
                    stage_s[:].rearrange("p (t b) -> p t b", t=FLUSH_),
                )
                nc.sync.dma_start(
                    out[1, :, c0:t + 1, :],
                    stage_v[:].rearrange("p (t b) -> p t b", t=FLUSH_),
                )


def _host_inputs(x, W1, b1, W2, b2, Bc=BC, T_=T, TB_=TB):
    """Per-core input dicts. x: [BATCH, T, F] fp32."""
    w1t = np.ascontiguousarray(
        W1.reshape(H, KC, P).transpose(2, 1, 0)).reshape(P, KC * H)
    w2t = np.ascontiguousarray(W2.T)
    u1i = np.ascontiguousarray(np.broadcast_to((70.0 - b1)[:, None], (H, Bc)))
    u2i = np.ascontiguousarray(np.broadcast_to((70.0 - b2)[:, None], (O, Bc)))
    g1 = np.ascontiguousarray((A_ * (85.0 - b1))[:, None])
    g2 = np.ascontiguousarray((A_ * (85.0 - b2))[:, None])
    n_cores = x.shape[0] // Bc
    in_maps = []
    for i in range(n_cores):
        xs = x[i * Bc:(i + 1) * Bc]  # [Bc, T, F]
        xTi = np.ascontiguousarray(
            xs.reshape(Bc, T_ // TB_, TB_, KC, P).transpose(1, 4, 2, 3, 0)
        ).reshape(T_ // TB_, P, TB_ * KC * Bc)
        in_maps.append({
            "xT": xTi, "w1t": w1t, "w2t": w2t,
            "u1i": u1i, "u2i": u2i, "g1": g1, "g2": g2,
        })
    return in_maps


def _install_ntff_shim():
    """Register the NTFF profile hook when the image's antenv lacks axon_hooks.

    Only needed for BASS_TRACE profiling runs; silently a no-op if anything
    is missing so plain correctness runs never depend on it.
    """
    import sys
    import types
    try:
        import antenv.axon_hooks  # noqa: F401  # already present: nothing to do
        return
    except ImportError:
        pass
    try:
        from trn_agent_boot.trn_boot import _ntff_profile_via_ctypes
        hook = _ntff_profile_via_ctypes("/opt/axon/libaxon_pjrt.so")
        mod = types.ModuleType("antenv.axon_hooks")
        mod._hook = hook
        mod.get_axon_ntff_profile_hook = lambda: mod._hook
        mod.set_axon_ntff_profile_hook = lambda h: setattr(mod, "_hook", h)
        sys.modules["antenv.axon_hooks"] = mod
    except Exception:
        pass


def kernel(x, W1, b1, W2, b2):
    global LAST_RUN
    if os.environ.get("BASS_TRACE"):
        _install_ntff_shim()
    x = np.ascontiguousarray(x, dtype=np.float32)
    W1 = np.asarray(W1, np.float32)
    b1 = np.asarray(b1, np.float32)
    W2 = np.asarray(W2, np.float32)
    b2 = np.asarray(b2, np.float32)

    nc = bacc.Bacc("TRN2", target_bir_lowering=False, debug=False,
                   num_devices=NCORES)
    with tile.TileContext(nc) as tc:
        with ExitStack() as ctx:
            build_program(nc, ctx, tc)
    nc.compile()

    in_maps = _host_inputs(x, W1, b1, W2, b2)
    res = run_bass_kernel_spmd(
        nc, in_maps, core_ids=list(range(NCORES)),
        trace=bool(os.environ.get("BASS_TRACE")),
    )
    LAST_RUN = res

    spk = np.empty((T, BATCH, O), np.float32)
    mem = np.empty((T, BATCH, O), np.float32)
    for i in range(NCORES):
        o = res.results[i]["out"]  # [2, O, T, Bc]
        spk[:, i * BC:(i + 1) * BC, :] = o[0].transpose(1, 2, 0)
        mem[:, i * BC:(i + 1) * BC, :] = o[1].transpose(1, 2, 0)
    return spk, mem

